# revision 11
# baseline (speedup 1.0000x reference)
"""DAriEL_Encoder_Cell_2 Trainium2 kernel (8-core SPMD, Bass/Tile), v2.

Reformulation of the reference:
  - Incremental LSTM: one real step per output step (7 steps; z_0 == 0).
  - 8-way tensor parallel over the 4*2048 gate columns (256 hidden/core).
  - Gate GEMM uses 4x PE column tiling: per K-chunk, 4 concurrent N=256
    matmuls land gates in PSUM as [128, 256] with partition = 32*j + b
    (hidden subslice j, batch b) and free = [i|f|o|g] x 64.  All cell
    elementwise then runs on 128 partitions with FD=64.
  - tanh/exp-only math (one activation table set): sigma(x) =
    (1+tanh(x/2))/2; cell state kept doubled (CC=2c, H=2h) with the 0.5
    folded into host-scaled Wh; exp(h) = exp(0.5*H) directly on ScalarE.
  - One AllGather per step (bf16): h^T [256,32] plus the 3 softmax
    partial stats ride together; z math is deferred one step so PE mode
    switches never stall the gate GEMM.
"""

import numpy as np

B, T, V, EMB, LAT = 32, 8, 2048, 256, 128
H = V
NC = 8
HS = H // NC               # 256 hidden units per core
GS = 4 * HS                # 1024 gate columns per core
KC = H // 128              # 16 contraction chunks
SUB = HS // 4              # 64 hidden units per column tile
# AG payload: two uniform 132-row blocks so every DMA stays 3-dim:
#   [0:128]   h^T rows 0..127     [128:131] stats  [131:132] pad
#   [132:260] h^T rows 128..255   [260:264] pad
AGR = 264
SIZE_LAT = 3.0

_CACHE = {}


def _build_program():
    import concourse.bacc as bacc
    import concourse.bass as bass
    import concourse.mybir as mybir
    import concourse.tile as tile

    f32 = mybir.dt.float32
    f32r = mybir.dt.float32r
    bf16 = mybir.dt.bfloat16
    i32 = mybir.dt.int32
    Alu = mybir.AluOpType
    Act = mybir.ActivationFunctionType

    nc = bacc.Bacc(
        "TRN2",
        target_bir_lowering=False,
        debug=False,
        num_devices=NC,
    )

    wh = nc.dram_tensor("wh", [H, GS], bf16, kind="ExternalInput")
    ewi = nc.dram_tensor("ewi", [V, GS], bf16, kind="ExternalInput")
    tok4 = nc.dram_tensor("tok4", [128, T], i32, kind="ExternalInput")
    iota = nc.dram_tensor("iota", [128, SUB], f32, kind="ExternalInput")
    sel4 = nc.dram_tensor("sel4", [128, B], f32, kind="ExternalInput")
    sel3 = nc.dram_tensor("sel3", [3 * NC, 2], bf16, kind="ExternalInput")
    idn32 = nc.dram_tensor("idn32", [B, B], bf16, kind="ExternalInput")
    idn128 = nc.dram_tensor("idn128", [128, 128], f32, kind="ExternalInput")
    zout = nc.dram_tensor("z", [B, T, LAT], f32, kind="ExternalOutput")

    agin = [
        nc.dram_tensor(f"agin{t}", [AGR, B], bf16, kind="Internal")
        for t in range(1, T)
    ]
    agout = [
        nc.dram_tensor(f"agout{t}", [AGR * NC, B], bf16, kind="Internal",
                       addr_space="Shared")
        for t in range(1, T)
    ]

    with tile.TileContext(nc) as tc:
        with (
            tc.tile_pool(name="const", bufs=1) as constp,
            tc.tile_pool(name="whp", bufs=1) as whp,
            tc.tile_pool(name="ewip", bufs=1) as ewip,
            tc.tile_pool(name="htsp", bufs=2) as htsp,
            tc.tile_pool(name="work", bufs=2) as workp,
            tc.tile_pool(name="state", bufs=2) as statep,
            tc.tile_pool(name="statsp", bufs=2) as statsp,
            tc.tile_pool(name="gpsum", bufs=2, space="PSUM") as gpsump,
            tc.tile_pool(name="spsum", bufs=1, space="PSUM") as spsump,
        ):
            # ---------------- prologue: constants + weights ----------------
            tok_sb = constp.tile([128, T], i32, tag="tok")
            nc.sync.dma_start(tok_sb[:], tok4[:])
            tokf = constp.tile([128, T], f32, tag="tokf")
            nc.vector.tensor_copy(tokf[:], tok_sb[:])

            iota_sb = constp.tile([128, SUB], f32, tag="iota")
            nc.sync.dma_start(iota_sb[:], iota[:])
            sel4_sb = constp.tile([128, B], f32, tag="sel4")
            nc.sync.dma_start(sel4_sb[:], sel4[:])
            sel3_sb = constp.tile([3 * NC, 2], bf16, tag="sel3")
            nc.sync.dma_start(sel3_sb[:], sel3[:])
            idn_sb = constp.tile([B, B], bf16, tag="idn")
            nc.sync.dma_start(idn_sb[:], idn32[:])
            idn128_sb = constp.tile([128, 128], f32, tag="idn128")
            nc.sync.dma_start(idn128_sb[:], idn128[:])

            # activation-table warmup: pulls exp_and_others during the
            # weight DMA instead of on the step-1 critical path
            warm = constp.tile([1, SUB], f32, tag="warm")
            nc.vector.memset(warm[:], 0.0)
            warm2 = constp.tile([1, SUB], f32, tag="warm2")
            nc.scalar.activation(warm2[:], warm[:], Act.Tanh)

            wh_sb = []
            for k in range(KC):
                wt = whp.tile([128, GS], bf16, tag=f"wh{k}", name=f"wh{k}")
                nc.sync.dma_start(wt[:], wh[k * 128:(k + 1) * 128, :])
                wh_sb.append(wt)

            ewi_sb = []
            for t in range(1, T):
                et = ewip.tile([B, GS], bf16, tag=f"ewi{t}", name=f"ewi{t}")
                nc.gpsimd.indirect_dma_start(
                    out=et[:],
                    out_offset=None,
                    in_=ewi[:],
                    in_offset=bass.IndirectOffsetOnAxis(
                        ap=tok_sb[0:B, t - 1:t], axis=0
                    ),
                )
                ewi_sb.append(et)

            zfull = constp.tile([B, T * LAT], f32, tag="zfull")
            nc.vector.memset(zfull[:, 0:LAT], 0.0)

            CC = None
            Hf = None
            hts = None
            stats_tiles = {}

            def emit_z(tt):
                ssb = stats_tiles[tt]
                pz = spsump.tile([1, 2 * B], f32, tag="pz", name=f"pz{tt}")
                nc.tensor.matmul(pz[0:1, 0:B], sel3_sb[:, 0:1], ssb[:],
                                 start=True, stop=True)
                nc.tensor.matmul(pz[0:1, B:2 * B], sel3_sb[:, 1:2], ssb[:],
                                 start=True, stop=True)
                zr = workp.tile([1, B], f32, tag="zr", name=f"zr{tt}")
                nc.vector.reciprocal(zr[:], pz[0:1, B:2 * B])
                zrow = workp.tile([1, B], f32, tag="zrow", name=f"zrow{tt}")
                nc.vector.tensor_mul(zrow[:], pz[0:1, 0:B], zr[:])
                zc = spsump.tile([B, 1], f32, tag="zc", name=f"zc{tt}")
                nc.tensor.transpose(zc[:], zrow[:], idn128_sb[0:1, 0:1])
                nc.vector.tensor_copy(
                    zfull[:, tt * LAT:(tt + 1) * LAT],
                    zc[:].to_broadcast([B, LAT]),
                )

            for t in range(1, T):
                # ---------------- gate GEMM (4x column-tiled) --------------
                G = gpsump.tile([128, 4 * SUB], f32, tag="G", name=f"G{t}")
                ew = ewi_sb[t - 1]
                for j in range(4):
                    nc.tensor.matmul(
                        G[32 * j:32 * j + 32, :],
                        idn_sb[:],
                        ew[:, 256 * j:256 * j + 256],
                        start=True, stop=(t == 1),
                        tile_position=(0, 32 * j),
                        skip_group_check=True,
                    )
                if t >= 2:
                    for k in range(KC):
                        last = (k == KC - 1)
                        for j in range(4):
                            nc.tensor.matmul(
                                G[32 * j:32 * j + 32, :],
                                hts[:, 32 * k:32 * k + 32],
                                wh_sb[k][:, 256 * j:256 * j + 256],
                                start=False, stop=last,
                                tile_position=(0, 32 * j),
                                skip_group_check=True,
                            )

                # ---------------- cell (tanh-only sigmoids) ----------------
                tifo = workp.tile([128, 3 * SUB], f32, tag="tifo",
                                  name=f"tifo{t}")
                nc.scalar.activation(tifo[:], G[:, 0:3 * SUB], Act.Tanh,
                                     scale=0.5)
                tg = workp.tile([128, SUB], f32, tag="tg", name=f"tg{t}")
                nc.scalar.activation(tg[:], G[:, 3 * SUB:4 * SUB], Act.Tanh)

                m_t = workp.tile([128, 1], f32, tag="mt", name=f"mt{t}")
                nc.vector.tensor_scalar(
                    m_t[:], tokf[:, t - 1:t], 0.0, None, Alu.is_gt
                )

                ti = tifo[:, 0:SUB]
                tf = tifo[:, SUB:2 * SUB]
                to = tifo[:, 2 * SUB:3 * SUB]

                v_ = workp.tile([128, SUB], f32, tag="v", name=f"v{t}")
                nc.vector.scalar_tensor_tensor(
                    out=v_[:], in0=ti, scalar=1.0, in1=tg[:],
                    op0=Alu.add, op1=Alu.mult,
                )
                CCn = statep.tile([128, SUB], f32, tag="cc", name=f"cc{t}")
                if t == 1:
                    nc.vector.tensor_scalar(
                        CCn[:], v_[:], m_t[:, 0:1], None, Alu.mult
                    )
                else:
                    u_ = workp.tile([128, SUB], f32, tag="u", name=f"u{t}")
                    nc.vector.scalar_tensor_tensor(
                        out=u_[:], in0=tf, scalar=1.0, in1=CC[:],
                        op0=Alu.add, op1=Alu.mult,
                    )
                    w_ = workp.tile([128, SUB], f32, tag="w", name=f"w{t}")
                    nc.vector.scalar_tensor_tensor(
                        out=w_[:], in0=u_[:], scalar=0.5, in1=v_[:],
                        op0=Alu.mult, op1=Alu.add,
                    )
                    dc = workp.tile([128, SUB], f32, tag="dc", name=f"dc{t}")
                    nc.vector.tensor_sub(dc[:], w_[:], CC[:])
                    nc.vector.scalar_tensor_tensor(
                        out=CCn[:], in0=dc[:], scalar=m_t[:, 0:1], in1=CC[:],
                        op0=Alu.mult, op1=Alu.add,
                    )
                CC = CCn

                tc_ = workp.tile([128, SUB], f32, tag="tc", name=f"tc{t}")
                nc.scalar.activation(tc_[:], CC[:], Act.Tanh, scale=0.5)
                Hn = workp.tile([128, SUB], f32, tag="hn", name=f"hn{t}")
                nc.vector.scalar_tensor_tensor(
                    out=Hn[:], in0=to, scalar=1.0, in1=tc_[:],
                    op0=Alu.add, op1=Alu.mult,
                )
                Hf2 = statep.tile([128, SUB], f32, tag="h", name=f"h{t}")
                if t == 1:
                    nc.vector.tensor_scalar(
                        Hf2[:], Hn[:], m_t[:, 0:1], None, Alu.mult
                    )
                else:
                    dh = workp.tile([128, SUB], f32, tag="dh", name=f"dh{t}")
                    nc.vector.tensor_sub(dh[:], Hn[:], Hf[:])
                    nc.vector.scalar_tensor_tensor(
                        out=Hf2[:], in0=dh[:], scalar=m_t[:, 0:1], in1=Hf[:],
                        op0=Alu.mult, op1=Alu.add,
                    )
                Hf = Hf2

                # ------------- ship h^T (only needed for t < 7) ------------
                if t < T - 1:
                    tp = spsump.tile([SUB, 128], f32, tag="tp", name=f"tp{t}")
                    nc.tensor.transpose(tp[:], Hf[:], idn128_sb[:])
                    hbT = workp.tile([SUB, 128], bf16, tag="hbT",
                                     name=f"hbT{t}")
                    nc.vector.tensor_copy(hbT[:], tp[:])
                    for kh in range(2):
                        nc.sync.dma_start(
                            agin[t - 1].ap()[132 * kh:132 * kh + 128, :]
                            .rearrange("(ph u) b -> u ph b", ph=2),
                            hbT[:, 64 * kh:64 * kh + 64]
                            .rearrange("u (ph b) -> u ph b", ph=2),
                        )

                # ------------- softmax partial stats -----------------------
                stk = statsp.tile([128, 4], f32, tag="stk", name=f"stk{t}")
                ex = workp.tile([128, SUB], f32, tag="ex", name=f"ex{t}")
                nc.scalar.activation(ex[:], Hf[:], Act.Exp, scale=0.5,
                                     accum_out=stk[:, 0:1])
                junk = workp.tile([128, SUB], f32, tag="junk", name=f"jk{t}")
                nc.vector.scalar_tensor_tensor(
                    out=junk[:], in0=iota_sb[:], scalar=tokf[:, t:t + 1],
                    in1=ex[:], op0=Alu.is_lt, op1=Alu.mult,
                    accum_out=stk[:, 1:2],
                )
                nc.vector.scalar_tensor_tensor(
                    out=junk[:], in0=iota_sb[:], scalar=tokf[:, t:t + 1],
                    in1=ex[:], op0=Alu.is_le, op1=Alu.mult,
                    accum_out=stk[:, 2:3],
                )
                pstat = spsump.tile([3, B], f32, tag="pstat",
                                    name=f"pstat{t}")
                nc.tensor.matmul(pstat[:], stk[:, 0:3], sel4_sb[:],
                                 start=True, stop=True)
                statb = workp.tile([3, B], bf16, tag="statb",
                                   name=f"statb{t}")
                nc.vector.tensor_copy(statb[:], pstat[:])
                nc.sync.dma_start(
                    agin[t - 1].ap()[128:131, :], statb[:]
                )

                # ------------- AllGather (h^T + stats together) ------------
                nc.gpsimd.collective_compute(
                    "AllGather",
                    Alu.bypass,
                    replica_groups=[list(range(NC))],
                    ins=[agin[t - 1].ap()],
                    outs=[agout[t - 1].ap()],
                )

                if t < T - 1:
                    # hts[p, 32k+b] = h^T global row 128k + p; gathered
                    # buffer is 16 uniform 132-row blocks (block = 2r + kh)
                    hts = htsp.tile([128, KC * B], bf16, tag="hts",
                                    name=f"hts{t}")
                    nc.sync.dma_start(
                        hts[:].rearrange("p (rk b) -> p rk b", rk=2 * NC),
                        agout[t - 1].ap()
                        .rearrange("(rk x) b -> x rk b", rk=2 * NC)
                        [0:128, :, :],
                    )
                ssb = statsp.tile([3 * NC, B], bf16, tag="ssb",
                                  name=f"ssb{t}")
                nc.sync.dma_start(
                    ssb[:],
                    agout[t - 1].ap()
                    .rearrange("(r row) b -> r row b", r=NC)
                    [:, 128:131, :],
                )
                stats_tiles[t] = ssb

                # z math deferred one step so its PE mode switches sit in
                # the post-GEMM idle window, never ahead of a gate GEMM
                if t >= 2:
                    emit_z(t - 1)

            emit_z(T - 1)

            # ---------------- epilogue: write z ----------------------------
            nc.sync.dma_start(
                zout.ap().rearrange("b t l -> b (t l)"), zfull[:]
            )

    nc.compile()
    return nc


def _prep_inputs(input_tokens, E, Wi, Wh, b):
    """Host-side sharding / weight fusion. Returns per-core input maps."""
    import ml_dtypes
    bf16 = ml_dtypes.bfloat16

    EWi = (E.astype(np.float64) @ Wi.astype(np.float64)
           + b.astype(np.float64))
    tok = np.ascontiguousarray(input_tokens.astype(np.int32))
    tok4 = np.ascontiguousarray(np.tile(tok, (4, 1)))      # [128, 8]
    idn = np.eye(B, dtype=np.float32)
    idn128 = np.eye(128, dtype=np.float32)

    # pstat[s, b] = sum_j stk[32j+b, s]
    sel4 = np.zeros((128, B), np.float32)
    p = np.arange(128)
    sel4[p, p % 32] = 1.0
    # pz[0, b] = 1.5*(num_lt+num_le), pz[1, b] = denom  (rows q = 3r + s)
    sel3 = np.zeros((3 * NC, 2), np.float32)
    q = np.arange(3 * NC)
    sel3[(q % 3 == 1) | (q % 3 == 2), 0] = 1.5
    sel3[q % 3 == 0, 1] = 1.0

    goff = (0, 2048, 6144, 4096)          # i, f, o, g  (Keras: i,f,g,o)
    in_maps = []
    for k in range(NC):
        base = k * HS
        cols = np.concatenate([
            np.arange(goff[g] + base + SUB * j, goff[g] + base + SUB * (j + 1))
            for j in range(4) for g in range(4)
        ])
        iota = np.zeros((128, SUB), np.float32)
        for j in range(4):
            iota[32 * j:32 * (j + 1), :] = (
                base + SUB * j + np.arange(SUB))[None, :]
        in_maps.append({
            "wh": np.ascontiguousarray((0.5 * Wh[:, cols]).astype(bf16)),
            "ewi": np.ascontiguousarray(EWi[:, cols].astype(bf16)),
            "tok4": tok4,
            "iota": iota,
            "sel4": sel4,
            "sel3": sel3.astype(bf16),
            "idn32": idn.astype(bf16),
            "idn128": idn128,
        })
    return in_maps


def kernel(input_tokens, E, Wi, Wh, b, _trace=False):
    from concourse import bass_utils

    if "nc" not in _CACHE:
        _CACHE["nc"] = _build_program()
    nc = _CACHE["nc"]

    in_maps = _prep_inputs(
        np.asarray(input_tokens), np.asarray(E), np.asarray(Wi),
        np.asarray(Wh), np.asarray(b),
    )
    res = bass_utils.run_bass_kernel_spmd(
        nc, in_maps, core_ids=list(range(NC)), trace=_trace,
    )
    _CACHE["last_result"] = res
    return np.asarray(res.results[0]["z"])


# revision 16
# speedup vs baseline: 1.0262x; 1.0262x over previous
"""DAriEL_Encoder_Cell_2 Trainium2 kernel (8-core SPMD, Bass/Tile), v2.

Reformulation of the reference:
  - Incremental LSTM: one real step per output step (7 steps; z_0 == 0).
  - 8-way tensor parallel over the 4*2048 gate columns (256 hidden/core).
  - Gate GEMM uses 4x PE column tiling: per K-chunk, 4 concurrent N=256
    matmuls land gates in PSUM as [128, 256] with partition = 32*j + b
    (hidden subslice j, batch b) and free = [i|f|o|g] x 64.  All cell
    elementwise then runs on 128 partitions with FD=64.
  - tanh/exp-only math (one activation table set): sigma(x) =
    (1+tanh(x/2))/2; cell state kept doubled (CC=2c, H=2h) with the 0.5
    folded into host-scaled Wh; exp(h) = exp(0.5*H) directly on ScalarE.
  - One AllGather per step (bf16): h^T [256,32] plus the 3 softmax
    partial stats ride together; z math is deferred one step so PE mode
    switches never stall the gate GEMM.
"""

import numpy as np

B, T, V, EMB, LAT = 32, 8, 2048, 256, 128
H = V
NC = 8
HS = H // NC               # 256 hidden units per core
GS = 4 * HS                # 1024 gate columns per core
KC = H // 128              # 16 contraction chunks
SUB = HS // 4              # 64 hidden units per column tile
AGR = 256                  # h^T AllGather rows (local hidden, in order)
SIZE_LAT = 3.0

_CACHE = {}


def _build_program():
    import concourse.bacc as bacc
    import concourse.bass as bass
    import concourse.mybir as mybir
    import concourse.tile as tile

    f32 = mybir.dt.float32
    f32r = mybir.dt.float32r
    bf16 = mybir.dt.bfloat16
    i32 = mybir.dt.int32
    Alu = mybir.AluOpType
    Act = mybir.ActivationFunctionType

    nc = bacc.Bacc(
        "TRN2",
        target_bir_lowering=False,
        debug=False,
        num_devices=NC,
    )

    wh = nc.dram_tensor("wh", [H, GS], bf16, kind="ExternalInput")
    ewi = nc.dram_tensor("ewi", [V, GS], bf16, kind="ExternalInput")
    tok4 = nc.dram_tensor("tok4", [128, T], i32, kind="ExternalInput")
    iota = nc.dram_tensor("iota", [128, SUB], f32, kind="ExternalInput")
    sel4 = nc.dram_tensor("sel4", [128, B], f32, kind="ExternalInput")
    sel3 = nc.dram_tensor("sel3", [3 * NC, 2], bf16, kind="ExternalInput")
    idn32 = nc.dram_tensor("idn32", [B, B], bf16, kind="ExternalInput")
    idn128 = nc.dram_tensor("idn128", [128, 128], f32, kind="ExternalInput")
    zout = nc.dram_tensor("z", [B, T, LAT], f32, kind="ExternalOutput")

    agin = [
        nc.dram_tensor(f"agin{t}", [AGR, B], bf16, kind="Internal")
        for t in range(1, T)
    ]
    agout = [
        nc.dram_tensor(f"agout{t}", [AGR * NC, B], bf16, kind="Internal",
                       addr_space="Shared")
        for t in range(1, T)
    ]
    agin2 = [
        nc.dram_tensor(f"agin2_{t}", [3, B], bf16, kind="Internal")
        for t in range(1, T)
    ]
    agout2 = [
        nc.dram_tensor(f"agout2_{t}", [3 * NC, B], bf16, kind="Internal",
                       addr_space="Shared")
        for t in range(1, T)
    ]

    with tile.TileContext(nc) as tc:
        with (
            tc.tile_pool(name="const", bufs=1) as constp,
            tc.tile_pool(name="whp", bufs=1) as whp,
            tc.tile_pool(name="ewip", bufs=1) as ewip,
            tc.tile_pool(name="htsp", bufs=2) as htsp,
            tc.tile_pool(name="work", bufs=2) as workp,
            tc.tile_pool(name="state", bufs=2) as statep,
            tc.tile_pool(name="statsp", bufs=2) as statsp,
            tc.tile_pool(name="gpsum", bufs=2, space="PSUM") as gpsump,
            tc.tile_pool(name="spsum", bufs=1, space="PSUM") as spsump,
        ):
            # ---------------- prologue: constants + weights ----------------
            tok_sb = constp.tile([128, T], i32, tag="tok")
            nc.sync.dma_start(tok_sb[:], tok4[:])
            tokf = constp.tile([128, T], f32, tag="tokf")
            nc.vector.tensor_copy(tokf[:], tok_sb[:])

            iota_sb = constp.tile([128, SUB], f32, tag="iota")
            nc.sync.dma_start(iota_sb[:], iota[:])
            sel4_sb = constp.tile([128, B], f32, tag="sel4")
            nc.sync.dma_start(sel4_sb[:], sel4[:])
            sel3_sb = constp.tile([3 * NC, 2], bf16, tag="sel3")
            nc.sync.dma_start(sel3_sb[:], sel3[:])
            idn_sb = constp.tile([B, B], bf16, tag="idn")
            nc.sync.dma_start(idn_sb[:], idn32[:])
            idn128_sb = constp.tile([128, 128], f32, tag="idn128")
            nc.sync.dma_start(idn128_sb[:], idn128[:])

            # activation-table warmup: pulls exp_and_others during the
            # weight DMA instead of on the step-1 critical path
            warm = constp.tile([1, SUB], f32, tag="warm")
            nc.vector.memset(warm[:], 0.0)
            warm2 = constp.tile([1, SUB], f32, tag="warm2")
            nc.scalar.activation(warm2[:], warm[:], Act.Tanh)

            wh_sb = []
            for k in range(KC):
                wt = whp.tile([128, GS], bf16, tag=f"wh{k}", name=f"wh{k}")
                nc.sync.dma_start(wt[:], wh[k * 128:(k + 1) * 128, :])
                wh_sb.append(wt)

            ewi_sb = []
            for t in range(1, T):
                et = ewip.tile([B, GS], bf16, tag=f"ewi{t}", name=f"ewi{t}")
                nc.gpsimd.indirect_dma_start(
                    out=et[:],
                    out_offset=None,
                    in_=ewi[:],
                    in_offset=bass.IndirectOffsetOnAxis(
                        ap=tok_sb[0:B, t - 1:t], axis=0
                    ),
                )
                ewi_sb.append(et)

            zfull = constp.tile([B, T * LAT], f32, tag="zfull")
            nc.vector.memset(zfull[:, 0:LAT], 0.0)

            CC = None
            Hf = None
            hts = None
            stats_tiles = {}

            def emit_z(tt):
                ssb = stats_tiles[tt]
                pz = spsump.tile([1, 2 * B], f32, tag="pz", name=f"pz{tt}")
                nc.tensor.matmul(pz[0:1, 0:B], sel3_sb[:, 0:1], ssb[:],
                                 start=True, stop=True)
                nc.tensor.matmul(pz[0:1, B:2 * B], sel3_sb[:, 1:2], ssb[:],
                                 start=True, stop=True)
                zr = workp.tile([1, B], f32, tag="zr", name=f"zr{tt}")
                nc.vector.reciprocal(zr[:], pz[0:1, B:2 * B])
                zrow = workp.tile([1, B], f32, tag="zrow", name=f"zrow{tt}")
                nc.vector.tensor_mul(zrow[:], pz[0:1, 0:B], zr[:])
                zc = spsump.tile([B, 1], f32, tag="zc", name=f"zc{tt}")
                nc.tensor.transpose(zc[:], zrow[:], idn128_sb[0:1, 0:1])
                nc.vector.tensor_copy(
                    zfull[:, tt * LAT:(tt + 1) * LAT],
                    zc[:].to_broadcast([B, LAT]),
                )

            for t in range(1, T):
                # ---------------- gate GEMM (4x column-tiled) --------------
                G = gpsump.tile([128, 4 * SUB], f32, tag="G", name=f"G{t}")
                ew = ewi_sb[t - 1]
                for j in range(4):
                    nc.tensor.matmul(
                        G[32 * j:32 * j + 32, :],
                        idn_sb[:],
                        ew[:, 256 * j:256 * j + 256],
                        start=True, stop=(t == 1),
                        tile_position=(0, 32 * j),
                        skip_group_check=True,
                    )
                if t >= 2:
                    for k in range(KC):
                        last = (k == KC - 1)
                        for j in range(4):
                            nc.tensor.matmul(
                                G[32 * j:32 * j + 32, :],
                                hts[:, 32 * k:32 * k + 32],
                                wh_sb[k][:, 256 * j:256 * j + 256],
                                start=False, stop=last,
                                tile_position=(0, 32 * j),
                                skip_group_check=True,
                            )

                # ---------------- cell (tanh-only sigmoids) ----------------
                tifo = workp.tile([128, 3 * SUB], f32, tag="tifo",
                                  name=f"tifo{t}")
                nc.scalar.activation(tifo[:], G[:, 0:3 * SUB], Act.Tanh,
                                     scale=0.5)
                tg = workp.tile([128, SUB], f32, tag="tg", name=f"tg{t}")
                nc.scalar.activation(tg[:], G[:, 3 * SUB:4 * SUB], Act.Tanh)

                m_t = workp.tile([128, 1], f32, tag="mt", name=f"mt{t}")
                nc.vector.tensor_scalar(
                    m_t[:], tokf[:, t - 1:t], 0.0, None, Alu.is_gt
                )

                ti = tifo[:, 0:SUB]
                tf = tifo[:, SUB:2 * SUB]
                to = tifo[:, 2 * SUB:3 * SUB]

                v_ = workp.tile([128, SUB], f32, tag="v", name=f"v{t}")
                nc.vector.scalar_tensor_tensor(
                    out=v_[:], in0=ti, scalar=1.0, in1=tg[:],
                    op0=Alu.add, op1=Alu.mult,
                )
                CCn = statep.tile([128, SUB], f32, tag="cc", name=f"cc{t}")
                if t == 1:
                    nc.vector.tensor_scalar(
                        CCn[:], v_[:], m_t[:, 0:1], None, Alu.mult
                    )
                else:
                    u_ = workp.tile([128, SUB], f32, tag="u", name=f"u{t}")
                    nc.vector.scalar_tensor_tensor(
                        out=u_[:], in0=tf, scalar=1.0, in1=CC[:],
                        op0=Alu.add, op1=Alu.mult,
                    )
                    w_ = workp.tile([128, SUB], f32, tag="w", name=f"w{t}")
                    nc.vector.scalar_tensor_tensor(
                        out=w_[:], in0=u_[:], scalar=0.5, in1=v_[:],
                        op0=Alu.mult, op1=Alu.add,
                    )
                    dc = workp.tile([128, SUB], f32, tag="dc", name=f"dc{t}")
                    nc.vector.tensor_sub(dc[:], w_[:], CC[:])
                    nc.vector.scalar_tensor_tensor(
                        out=CCn[:], in0=dc[:], scalar=m_t[:, 0:1], in1=CC[:],
                        op0=Alu.mult, op1=Alu.add,
                    )
                CC = CCn

                tc_ = workp.tile([128, SUB], f32, tag="tc", name=f"tc{t}")
                nc.scalar.activation(tc_[:], CC[:], Act.Tanh, scale=0.5)
                Hn = workp.tile([128, SUB], f32, tag="hn", name=f"hn{t}")
                nc.vector.scalar_tensor_tensor(
                    out=Hn[:], in0=to, scalar=1.0, in1=tc_[:],
                    op0=Alu.add, op1=Alu.mult,
                )
                Hf2 = statep.tile([128, SUB], f32, tag="h", name=f"h{t}")
                if t == 1:
                    nc.vector.tensor_scalar(
                        Hf2[:], Hn[:], m_t[:, 0:1], None, Alu.mult
                    )
                else:
                    dh = workp.tile([128, SUB], f32, tag="dh", name=f"dh{t}")
                    nc.vector.tensor_sub(dh[:], Hn[:], Hf[:])
                    nc.vector.scalar_tensor_tensor(
                        out=Hf2[:], in0=dh[:], scalar=m_t[:, 0:1], in1=Hf[:],
                        op0=Alu.mult, op1=Alu.add,
                    )
                Hf = Hf2

                # ------------- ship h^T + h AllGather (t < 7) --------------
                if t < T - 1:
                    tp = spsump.tile([SUB, 128], f32, tag="tp", name=f"tp{t}")
                    nc.tensor.transpose(tp[:], Hf[:], idn128_sb[:])
                    hbT = workp.tile([SUB, 128], bf16, tag="hbT",
                                     name=f"hbT{t}")
                    nc.vector.tensor_copy(hbT[:], tp[:])
                    nc.sync.dma_start(
                        agin[t - 1].ap().rearrange("(j u) b -> u j b", j=4),
                        hbT[:].rearrange("u (j b) -> u j b", j=4),
                    )
                    nc.gpsimd.collective_compute(
                        "AllGather",
                        Alu.bypass,
                        replica_groups=[list(range(NC))],
                        ins=[agin[t - 1].ap()],
                        outs=[agout[t - 1].ap()],
                    )

                # ------------- softmax partial stats -----------------------
                stk = statsp.tile([128, 4], f32, tag="stk", name=f"stk{t}")
                ex = workp.tile([128, SUB], f32, tag="ex", name=f"ex{t}")
                nc.scalar.activation(ex[:], Hf[:], Act.Exp, scale=0.5,
                                     accum_out=stk[:, 0:1])
                junk = workp.tile([128, SUB], f32, tag="junk", name=f"jk{t}")
                nc.vector.scalar_tensor_tensor(
                    out=junk[:], in0=iota_sb[:], scalar=tokf[:, t:t + 1],
                    in1=ex[:], op0=Alu.is_lt, op1=Alu.mult,
                    accum_out=stk[:, 1:2],
                )
                nc.vector.scalar_tensor_tensor(
                    out=junk[:], in0=iota_sb[:], scalar=tokf[:, t:t + 1],
                    in1=ex[:], op0=Alu.is_le, op1=Alu.mult,
                    accum_out=stk[:, 2:3],
                )
                pstat = spsump.tile([3, B], f32, tag="pstat",
                                    name=f"pstat{t}")
                nc.tensor.matmul(pstat[:], stk[:, 0:3], sel4_sb[:],
                                 start=True, stop=True)
                statb = workp.tile([3, B], bf16, tag="statb",
                                   name=f"statb{t}")
                nc.vector.tensor_copy(statb[:], pstat[:])
                nc.scalar.dma_start(agin2[t - 1].ap(), statb[:])
                nc.gpsimd.collective_compute(
                    "AllGather",
                    Alu.bypass,
                    replica_groups=[list(range(NC))],
                    ins=[agin2[t - 1].ap()],
                    outs=[agout2[t - 1].ap()],
                )

                # z math deferred one step so its PE mode switches and DVE
                # ops sit in the post-GEMM idle window
                if t >= 2:
                    emit_z(t - 1)

                if t < T - 1:
                    # hts[p, 32k+b] = h^T global row 128k + p
                    #              = agout[256*(k//2) + 128*(k%2) + p, b];
                    # two half DMAs on separate HWDGE queues
                    hts = htsp.tile([128, KC * B], bf16, tag="hts",
                                    name=f"hts{t}")
                    src = agout[t - 1].ap().rearrange(
                        "(r x) b -> x r b", r=NC)
                    dst = hts[:].rearrange("p (r x) -> p r x", r=NC)
                    nc.sync.dma_start(
                        dst[:, :, 0:B], src[0:128, :, :],
                    )
                    nc.scalar.dma_start(
                        dst[:, :, B:2 * B], src[128:256, :, :],
                    )
                ssb = statsp.tile([3 * NC, B], bf16, tag="ssb",
                                  name=f"ssb{t}")
                nc.sync.dma_start(ssb[:], agout2[t - 1].ap())
                stats_tiles[t] = ssb

            emit_z(T - 1)

            # ---------------- epilogue: write z ----------------------------
            nc.sync.dma_start(
                zout.ap().rearrange("b t l -> b (t l)"), zfull[:]
            )

    nc.compile()
    return nc


def _prep_inputs(input_tokens, E, Wi, Wh, b):
    """Host-side sharding / weight fusion. Returns per-core input maps."""
    import ml_dtypes
    bf16 = ml_dtypes.bfloat16

    EWi = (E.astype(np.float64) @ Wi.astype(np.float64)
           + b.astype(np.float64))
    tok = np.ascontiguousarray(input_tokens.astype(np.int32))
    tok4 = np.ascontiguousarray(np.tile(tok, (4, 1)))      # [128, 8]
    idn = np.eye(B, dtype=np.float32)
    idn128 = np.eye(128, dtype=np.float32)

    # pstat[s, b] = sum_j stk[32j+b, s]
    sel4 = np.zeros((128, B), np.float32)
    p = np.arange(128)
    sel4[p, p % 32] = 1.0
    # pz[0, b] = 1.5*(num_lt+num_le), pz[1, b] = denom  (rows q = 3r + s)
    sel3 = np.zeros((3 * NC, 2), np.float32)
    q = np.arange(3 * NC)
    sel3[(q % 3 == 1) | (q % 3 == 2), 0] = 1.5
    sel3[q % 3 == 0, 1] = 1.0

    goff = (0, 2048, 6144, 4096)          # i, f, o, g  (Keras: i,f,g,o)
    in_maps = []
    for k in range(NC):
        base = k * HS
        cols = np.concatenate([
            np.arange(goff[g] + base + SUB * j, goff[g] + base + SUB * (j + 1))
            for j in range(4) for g in range(4)
        ])
        iota = np.zeros((128, SUB), np.float32)
        for j in range(4):
            iota[32 * j:32 * (j + 1), :] = (
                base + SUB * j + np.arange(SUB))[None, :]
        in_maps.append({
            "wh": np.ascontiguousarray((0.5 * Wh[:, cols]).astype(bf16)),
            "ewi": np.ascontiguousarray(EWi[:, cols].astype(bf16)),
            "tok4": tok4,
            "iota": iota,
            "sel4": sel4,
            "sel3": sel3.astype(bf16),
            "idn32": idn.astype(bf16),
            "idn128": idn128,
        })
    return in_maps


def kernel(input_tokens, E, Wi, Wh, b, _trace=False):
    from concourse import bass_utils

    if "nc" not in _CACHE:
        _CACHE["nc"] = _build_program()
    nc = _CACHE["nc"]

    in_maps = _prep_inputs(
        np.asarray(input_tokens), np.asarray(E), np.asarray(Wi),
        np.asarray(Wh), np.asarray(b),
    )
    res = bass_utils.run_bass_kernel_spmd(
        nc, in_maps, core_ids=list(range(NC)), trace=_trace,
    )
    _CACHE["last_result"] = res
    return np.asarray(res.results[0]["z"])


# revision 19
# speedup vs baseline: 1.0644x; 1.0373x over previous
"""DAriEL_Encoder_Cell_2 Trainium2 kernel (8-core SPMD, Bass/Tile), v2.

Reformulation of the reference:
  - Incremental LSTM: one real step per output step (7 steps; z_0 == 0).
  - 8-way tensor parallel over the 4*2048 gate columns (256 hidden/core).
  - Gate GEMM uses 4x PE column tiling: per K-chunk, 4 concurrent N=256
    matmuls land gates in PSUM as [128, 256] with partition = 32*j + b
    (hidden subslice j, batch b) and free = [i|f|o|g] x 64.  All cell
    elementwise then runs on 128 partitions with FD=64.
  - tanh/exp-only math (one activation table set): sigma(x) =
    (1+tanh(x/2))/2; cell state kept doubled (CC=2c, H=2h) with the 0.5
    folded into host-scaled Wh; exp(h) = exp(0.5*H) directly on ScalarE.
  - One AllGather per step (bf16): h^T [256,32] plus the 3 softmax
    partial stats ride together; z math is deferred one step so PE mode
    switches never stall the gate GEMM.
"""

import numpy as np

B, T, V, EMB, LAT = 32, 8, 2048, 256, 128
H = V
NC = 8
HS = H // NC               # 256 hidden units per core
GS = 4 * HS                # 1024 gate columns per core
KC = H // 128              # 16 contraction chunks
SUB = HS // 4              # 64 hidden units per column tile
AGR = 256                  # h^T AllGather rows (local hidden, in order)
SIZE_LAT = 3.0

_CACHE = {}


def _build_program(mask_steps):
    import concourse.bacc as bacc
    import concourse.bass as bass
    import concourse.mybir as mybir
    import concourse.tile as tile

    f32 = mybir.dt.float32
    f32r = mybir.dt.float32r
    bf16 = mybir.dt.bfloat16
    i32 = mybir.dt.int32
    Alu = mybir.AluOpType
    Act = mybir.ActivationFunctionType

    nc = bacc.Bacc(
        "TRN2",
        target_bir_lowering=False,
        debug=False,
        num_devices=NC,
    )

    wh = nc.dram_tensor("wh", [H, GS], bf16, kind="ExternalInput")
    ewi = nc.dram_tensor("ewi", [V, GS], bf16, kind="ExternalInput")
    tok4 = nc.dram_tensor("tok4", [128, T], i32, kind="ExternalInput")
    # packed constants: fpack = [iota | sel4 | idn128], bpack = [idn32 | sel3]
    fpack = nc.dram_tensor("fpack", [128, SUB + B + 128], f32,
                           kind="ExternalInput")
    bpack = nc.dram_tensor("bpack", [B, B + 2], bf16, kind="ExternalInput")
    zout = nc.dram_tensor("z", [B, T, LAT], f32, kind="ExternalOutput")

    agin = [
        nc.dram_tensor(f"agin{t}", [AGR, B], bf16, kind="Internal")
        for t in range(1, T)
    ]
    agout = [
        nc.dram_tensor(f"agout{t}", [AGR * NC, B], bf16, kind="Internal",
                       addr_space="Shared")
        for t in range(1, T)
    ]
    agin2 = [
        nc.dram_tensor(f"agin2_{t}", [3, B], bf16, kind="Internal")
        for t in range(1, T)
    ]
    agout2 = [
        nc.dram_tensor(f"agout2_{t}", [3 * NC, B], bf16, kind="Internal",
                       addr_space="Shared")
        for t in range(1, T)
    ]

    with tile.TileContext(nc) as tc:
        with (
            tc.tile_pool(name="const", bufs=1) as constp,
            tc.tile_pool(name="whp", bufs=1) as whp,
            tc.tile_pool(name="ewip", bufs=1) as ewip,
            tc.tile_pool(name="htsp", bufs=2) as htsp,
            tc.tile_pool(name="work", bufs=2) as workp,
            tc.tile_pool(name="state", bufs=2) as statep,
            tc.tile_pool(name="statsp", bufs=2) as statsp,
            tc.tile_pool(name="gpsum", bufs=2, space="PSUM") as gpsump,
            tc.tile_pool(name="spsum", bufs=1, space="PSUM") as spsump,
        ):
            # ---------------- prologue: constants + weights ----------------
            tok_sb = constp.tile([128, T], i32, tag="tok")
            nc.sync.dma_start(tok_sb[:], tok4[:])
            tokf = constp.tile([128, T], f32, tag="tokf")
            nc.vector.tensor_copy(tokf[:], tok_sb[:])

            fp_sb = constp.tile([128, SUB + B + 128], f32, tag="fpack")
            nc.sync.dma_start(fp_sb[:], fpack[:])
            iota_sb = fp_sb[:, 0:SUB]
            sel4_sb = fp_sb[:, SUB:SUB + B]
            idn128_sb = fp_sb[:, SUB + B:SUB + B + 128]
            bp_sb = constp.tile([B, B + 2], bf16, tag="bpack")
            nc.sync.dma_start(bp_sb[:], bpack[:])
            idn_sb = bp_sb[:, 0:B]
            sel3_sb = bp_sb[0:3 * NC, B:B + 2]

            # activation-table warmup: pulls exp_and_others during the
            # weight DMA instead of on the step-1 critical path
            warm = constp.tile([1, SUB], f32, tag="warm")
            nc.vector.memset(warm[:], 0.0)
            warm2 = constp.tile([1, SUB], f32, tag="warm2")
            nc.scalar.activation(warm2[:], warm[:], Act.Tanh)

            wh_sb = []
            for k in range(KC):
                wt = whp.tile([128, GS], bf16, tag=f"wh{k}", name=f"wh{k}")
                nc.scalar.dma_start(wt[:], wh[k * 128:(k + 1) * 128, :])
                wh_sb.append(wt)

            def emit_gather(t):
                et = ewip.tile([B, GS], bf16, tag=f"ewi{t}", name=f"ewi{t}")
                nc.gpsimd.indirect_dma_start(
                    out=et[:],
                    out_offset=None,
                    in_=ewi[:],
                    in_offset=bass.IndirectOffsetOnAxis(
                        ap=tok_sb[0:B, t - 1:t], axis=0
                    ),
                )
                return et

            ewi_sb = {1: emit_gather(1)}

            zfull = constp.tile([B, T * LAT], f32, tag="zfull")
            nc.vector.memset(zfull[:, 0:LAT], 0.0)

            CC = None
            Hf = None
            hts = None
            stats_tiles = {}

            def emit_z(tt):
                ssb = stats_tiles[tt]
                pz = spsump.tile([1, 2 * B], f32, tag="pz", name=f"pz{tt}")
                nc.tensor.matmul(pz[0:1, 0:B], sel3_sb[:, 0:1], ssb[:],
                                 start=True, stop=True)
                nc.tensor.matmul(pz[0:1, B:2 * B], sel3_sb[:, 1:2], ssb[:],
                                 start=True, stop=True)
                zr = workp.tile([1, B], f32, tag="zr", name=f"zr{tt}")
                nc.vector.reciprocal(zr[:], pz[0:1, B:2 * B])
                zrow = workp.tile([1, B], f32, tag="zrow", name=f"zrow{tt}")
                nc.vector.tensor_mul(zrow[:], pz[0:1, 0:B], zr[:])
                zc = spsump.tile([B, 1], f32, tag="zc", name=f"zc{tt}")
                nc.tensor.transpose(zc[:], zrow[:], idn128_sb[0:1, 0:1])
                nc.vector.tensor_copy(
                    zfull[:, tt * LAT:(tt + 1) * LAT],
                    zc[:].to_broadcast([B, LAT]),
                )

            for t in range(1, T):
                # ---------------- gate GEMM (4x column-tiled) --------------
                G = gpsump.tile([128, 4 * SUB], f32, tag="G", name=f"G{t}")
                ew = ewi_sb[t]
                for j in range(4):
                    nc.tensor.matmul(
                        G[32 * j:32 * j + 32, :],
                        idn_sb,
                        ew[:, 256 * j:256 * j + 256],
                        start=True, stop=(t == 1),
                        tile_position=(0, 32 * j),
                        skip_group_check=True,
                    )
                if t >= 2:
                    for k in range(KC):
                        last = (k == KC - 1)
                        for j in range(4):
                            nc.tensor.matmul(
                                G[32 * j:32 * j + 32, :],
                                hts[:, 32 * k:32 * k + 32],
                                wh_sb[k][:, 256 * j:256 * j + 256],
                                start=False, stop=last,
                                tile_position=(0, 32 * j),
                                skip_group_check=True,
                            )

                # ---------------- cell (tanh-only sigmoids) ----------------
                tifo = workp.tile([128, 3 * SUB], f32, tag="tifo",
                                  name=f"tifo{t}")
                nc.scalar.activation(tifo[:], G[:, 0:3 * SUB], Act.Tanh,
                                     scale=0.5)
                tg = workp.tile([128, SUB], f32, tag="tg", name=f"tg{t}")
                nc.scalar.activation(tg[:], G[:, 3 * SUB:4 * SUB], Act.Tanh)

                # mask ops are compiled only for steps whose token
                # column actually contains PAD (host-keyed specialization)
                need_m = mask_steps[t - 1]
                if need_m:
                    m_t = workp.tile([128, 1], f32, tag="mt", name=f"mt{t}")
                    nc.vector.tensor_scalar(
                        m_t[:], tokf[:, t - 1:t], 0.0, None, Alu.is_gt
                    )

                ti = tifo[:, 0:SUB]
                tf = tifo[:, SUB:2 * SUB]
                to = tifo[:, 2 * SUB:3 * SUB]

                v_ = workp.tile([128, SUB], f32, tag="v", name=f"v{t}")
                nc.vector.scalar_tensor_tensor(
                    out=v_[:], in0=ti, scalar=1.0, in1=tg[:],
                    op0=Alu.add, op1=Alu.mult,
                )
                if t == 1:
                    if need_m:
                        CCn = statep.tile([128, SUB], f32, tag="cc",
                                          name=f"cc{t}")
                        nc.vector.tensor_scalar(
                            CCn[:], v_[:], m_t[:, 0:1], None, Alu.mult
                        )
                    else:
                        CCn = v_
                else:
                    u_ = workp.tile([128, SUB], f32, tag="u", name=f"u{t}")
                    nc.vector.scalar_tensor_tensor(
                        out=u_[:], in0=tf, scalar=1.0, in1=CC[:],
                        op0=Alu.add, op1=Alu.mult,
                    )
                    w_ = statep.tile([128, SUB], f32, tag="cc", name=f"cc{t}")
                    nc.vector.scalar_tensor_tensor(
                        out=w_[:], in0=u_[:], scalar=0.5, in1=v_[:],
                        op0=Alu.mult, op1=Alu.add,
                    )
                    if need_m:
                        dc = workp.tile([128, SUB], f32, tag="dc",
                                        name=f"dc{t}")
                        nc.vector.tensor_sub(dc[:], w_[:], CC[:])
                        CCn = statep.tile([128, SUB], f32, tag="cc2",
                                          name=f"cc2{t}")
                        nc.vector.scalar_tensor_tensor(
                            out=CCn[:], in0=dc[:], scalar=m_t[:, 0:1],
                            in1=CC[:], op0=Alu.mult, op1=Alu.add,
                        )
                    else:
                        CCn = w_
                CC = CCn

                tc_ = workp.tile([128, SUB], f32, tag="tc", name=f"tc{t}")
                nc.scalar.activation(tc_[:], CC[:], Act.Tanh, scale=0.5)
                Hn = statep.tile([128, SUB], f32, tag="h", name=f"hn{t}")
                nc.vector.scalar_tensor_tensor(
                    out=Hn[:], in0=to, scalar=1.0, in1=tc_[:],
                    op0=Alu.add, op1=Alu.mult,
                )
                if not need_m:
                    Hf2 = Hn
                elif t == 1:
                    Hf2 = statep.tile([128, SUB], f32, tag="h2",
                                      name=f"h{t}")
                    nc.vector.tensor_scalar(
                        Hf2[:], Hn[:], m_t[:, 0:1], None, Alu.mult
                    )
                else:
                    dh = workp.tile([128, SUB], f32, tag="dh", name=f"dh{t}")
                    nc.vector.tensor_sub(dh[:], Hn[:], Hf[:])
                    Hf2 = statep.tile([128, SUB], f32, tag="h2",
                                      name=f"h{t}")
                    nc.vector.scalar_tensor_tensor(
                        out=Hf2[:], in0=dh[:], scalar=m_t[:, 0:1], in1=Hf[:],
                        op0=Alu.mult, op1=Alu.add,
                    )
                Hf = Hf2

                # ------------- ship h^T + h AllGather (t < 7) --------------
                if t < T - 1:
                    tp = spsump.tile([SUB, 128], f32, tag="tp", name=f"tp{t}")
                    nc.tensor.transpose(tp[:], Hf[:], idn128_sb)
                    hbT = workp.tile([SUB, 128], bf16, tag="hbT",
                                     name=f"hbT{t}")
                    nc.vector.tensor_copy(hbT[:], tp[:])
                    nc.sync.dma_start(
                        agin[t - 1].ap().rearrange("(j u) b -> u j b", j=4),
                        hbT[:].rearrange("u (j b) -> u j b", j=4),
                    )
                    nc.gpsimd.collective_compute(
                        "AllGather",
                        Alu.bypass,
                        replica_groups=[list(range(NC))],
                        ins=[agin[t - 1].ap()],
                        outs=[agout[t - 1].ap()],
                    )

                # ------------- softmax partial stats -----------------------
                stk = statsp.tile([128, 4], f32, tag="stk", name=f"stk{t}")
                ex = workp.tile([128, SUB], f32, tag="ex", name=f"ex{t}")
                nc.scalar.activation(ex[:], Hf[:], Act.Exp, scale=0.5,
                                     accum_out=stk[:, 0:1])
                junk = workp.tile([128, SUB], f32, tag="junk", name=f"jk{t}")
                nc.vector.scalar_tensor_tensor(
                    out=junk[:], in0=iota_sb, scalar=tokf[:, t:t + 1],
                    in1=ex[:], op0=Alu.is_lt, op1=Alu.mult,
                    accum_out=stk[:, 1:2],
                )
                nc.vector.scalar_tensor_tensor(
                    out=junk[:], in0=iota_sb, scalar=tokf[:, t:t + 1],
                    in1=ex[:], op0=Alu.is_le, op1=Alu.mult,
                    accum_out=stk[:, 2:3],
                )
                pstat = spsump.tile([3, B], f32, tag="pstat",
                                    name=f"pstat{t}")
                nc.tensor.matmul(pstat[:], stk[:, 0:3], sel4_sb,
                                 start=True, stop=True)
                statb = workp.tile([3, B], bf16, tag="statb",
                                   name=f"statb{t}")
                nc.vector.tensor_copy(statb[:], pstat[:])
                nc.scalar.dma_start(agin2[t - 1].ap(), statb[:])
                nc.gpsimd.collective_compute(
                    "AllGather",
                    Alu.bypass,
                    replica_groups=[list(range(NC))],
                    ins=[agin2[t - 1].ap()],
                    outs=[agout2[t - 1].ap()],
                )

                # z math deferred one step so its PE mode switches and DVE
                # ops sit in the post-GEMM idle window
                if t >= 2:
                    emit_z(t - 1)

                if t < T - 1:
                    # hts[p, 32k+b] = h^T global row 128k + p
                    #              = agout[256*(k//2) + 128*(k%2) + p, b];
                    # two half DMAs on separate HWDGE queues
                    hts = htsp.tile([128, KC * B], bf16, tag="hts",
                                    name=f"hts{t}")
                    src = agout[t - 1].ap().rearrange(
                        "(r x) b -> x r b", r=NC)
                    dst = hts[:].rearrange("p (r x) -> p r x", r=NC)
                    nc.sync.dma_start(
                        dst[:, :, 0:B], src[0:128, :, :],
                    )
                    nc.scalar.dma_start(
                        dst[:, :, B:2 * B], src[128:256, :, :],
                    )
                ssb = statsp.tile([3 * NC, B], bf16, tag="ssb",
                                  name=f"ssb{t}")
                nc.sync.dma_start(ssb[:], agout2[t - 1].ap())
                stats_tiles[t] = ssb
                if t + 1 < T:
                    ewi_sb[t + 1] = emit_gather(t + 1)

            emit_z(T - 1)

            # ---------------- epilogue: write z ----------------------------
            nc.sync.dma_start(
                zout.ap().rearrange("b t l -> b (t l)"), zfull[:]
            )

    nc.compile()
    return nc


def _prep_inputs(input_tokens, E, Wi, Wh, b):
    """Host-side sharding / weight fusion. Returns per-core input maps."""
    import ml_dtypes
    bf16 = ml_dtypes.bfloat16

    EWi = (E.astype(np.float64) @ Wi.astype(np.float64)
           + b.astype(np.float64))
    tok = np.ascontiguousarray(input_tokens.astype(np.int32))
    tok4 = np.ascontiguousarray(np.tile(tok, (4, 1)))      # [128, 8]
    idn = np.eye(B, dtype=np.float32)
    idn128 = np.eye(128, dtype=np.float32)

    # pstat[s, b] = sum_j stk[32j+b, s]
    sel4 = np.zeros((128, B), np.float32)
    p = np.arange(128)
    sel4[p, p % 32] = 1.0
    # pz[0, b] = 1.5*(num_lt+num_le), pz[1, b] = denom  (rows q = 3r + s)
    sel3 = np.zeros((3 * NC, 2), np.float32)
    q = np.arange(3 * NC)
    sel3[(q % 3 == 1) | (q % 3 == 2), 0] = 1.5
    sel3[q % 3 == 0, 1] = 1.0
    bpack = np.zeros((B, B + 2), np.float32)
    bpack[:, 0:B] = idn
    bpack[0:3 * NC, B:B + 2] = sel3
    bpack = bpack.astype(bf16)

    goff = (0, 2048, 6144, 4096)          # i, f, o, g  (Keras: i,f,g,o)
    in_maps = []
    for k in range(NC):
        base = k * HS
        cols = np.concatenate([
            np.arange(goff[g] + base + SUB * j, goff[g] + base + SUB * (j + 1))
            for j in range(4) for g in range(4)
        ])
        iota = np.zeros((128, SUB), np.float32)
        for j in range(4):
            iota[32 * j:32 * (j + 1), :] = (
                base + SUB * j + np.arange(SUB))[None, :]
        fpack = np.concatenate([iota, sel4, idn128], axis=1)
        in_maps.append({
            "wh": np.ascontiguousarray((0.5 * Wh[:, cols]).astype(bf16)),
            "ewi": np.ascontiguousarray(EWi[:, cols].astype(bf16)),
            "tok4": tok4,
            "fpack": np.ascontiguousarray(fpack),
            "bpack": np.ascontiguousarray(bpack),
        })
    return in_maps


def kernel(input_tokens, E, Wi, Wh, b, _trace=False):
    from concourse import bass_utils

    tok_arr = np.asarray(input_tokens)
    mask_steps = tuple(bool((tok_arr[:, t] == 0).any()) for t in range(T))
    key = ("nc", mask_steps)
    if key not in _CACHE:
        _CACHE[key] = _build_program(mask_steps)
    nc = _CACHE[key]

    in_maps = _prep_inputs(
        np.asarray(input_tokens), np.asarray(E), np.asarray(Wi),
        np.asarray(Wh), np.asarray(b),
    )
    res = bass_utils.run_bass_kernel_spmd(
        nc, in_maps, core_ids=list(range(NC)), trace=_trace,
    )
    _CACHE["last_result"] = res
    return np.asarray(res.results[0]["z"])
